# revision 2
# baseline (speedup 1.0000x reference)
"""BailingMoE (top-4 of 16 experts + shared expert) on 8 Trainium2 NeuronCores.

Strategy (expert-parallel, sparse dispatch V4):
  - The router (67 MFLOP, 0.01% of total work) runs on the host in exact fp32;
    tokens are dispatched host-side into per-expert compact buffers (the
    all-to-all "dispatch" leg of the expert-parallel recipe; the combine leg
    is the host scatter-add, matching the baseline's host all-reduce of the
    shared partials).
  - Each core owns 2 experts and 1/8 of the shared-expert intermediate dim.
    Experts are paired big-load + small-load onto cores so the two SPMD slot
    capacities C0 >= C1 hug the actual token loads (~4x less routed FLOPs
    than dense).
  - Everything runs in bf16 (1 PE cycle/row, half the HBM bytes of fp32);
    PSUM accumulation is fp32. Measured end-to-end rel err ~4e-3 (gate 2e-2).
  - The kernel is DMA/PE co-limited (~46MB in vs ~190us of PE), so all
    activation I/O is packed as [128, KH*C] monoliths whose DMA descriptors
    are 9-32KB per partition row (full HBM bandwidth), and weight streams are
    emitted in exact consumption order with deep prefetch rings.
  - Outputs: per-expert compact y^T (bf16) + the core's shared partial y^T
    (bf16). Host applies the renormalized top-4 combine weights during the
    scatter-add in fp32.
"""

import numpy as np
from ml_dtypes import bfloat16

import concourse.bass as bass
import concourse.mybir as mybir
import concourse.tile as tile
from concourse.bass_utils import run_bass_kernel_spmd

# ---------------------------------------------------------------------------
# Walrus in this container rejects >1 sem-wait condition per instruction
# ("Too many sync wait commands"). Engines run their streams in order, so
# excess waits are legal on same-engine NoOps inserted before the instruction.
# ---------------------------------------------------------------------------
_counter = [0]


def _make_wait_nop(template_inst, waits):
    _counter[0] += 1
    nop = mybir.InstNoOp(
        name=f"I-waitsplit-{_counter[0]}", ins=[], outs=[], bass_nofuse=True
    )
    nop.engine = template_inst.engine
    nop.debug = template_inst.debug
    nop.sync_info = mybir.SyncInfo(on_wait=list(waits), on_update=[])
    return nop


def _split_all_waits(nc):
    for bass_bb in nc.bb_map.values():
        insts = bass_bb.bb.instructions
        i = 0
        while i < len(insts):
            inst = insts[i]
            si = inst.sync_info
            if si is not None and len(si.on_wait) > 1:
                waits = list(si.on_wait)
                del si.on_wait[:]
                si.on_wait.append(waits[-1])
                for j, w in enumerate(waits[:-1]):
                    nop = _make_wait_nop(inst, [w])
                    nc.register_instruction(nop, overwrite=True)
                    insts.insert(i + j, nop)
                i += len(waits) - 1
            i += 1


_PATCHED = [False]


def _install_cc_hook_debug():
    """Surface compile-hook exceptions (PJRT reports them as an opaque
    CallFunctionObjArgs error otherwise)."""
    import traceback
    import concourse.bass2jax as b2j
    b2j.install_neuronx_cc_hook()
    try:
        import libneuronxla
    except ImportError:
        return
    if getattr(libneuronxla, "_kernel_dbg_wrapped", False):
        return
    real = libneuronxla.neuronx_cc

    def hook(*a, **k):
        try:
            return real(*a, **k)
        except BaseException:
            traceback.print_exc()
            raise

    libneuronxla.neuronx_cc = hook
    libneuronxla._kernel_dbg_wrapped = True
    b2j.install_neuronx_cc_hook = lambda: None


def _apply_tile_patch():
    if _PATCHED[0]:
        return
    _PATCHED[0] = True
    _install_cc_hook_debug()

    def _drain_and_barrier(self, tick_clock, wait_clock):
        nc = self.nc
        drain_inst = nc.sync.drain()
        wait_clock.add_sem_waits(
            drain_inst.ins, tile.ScopedClock({None: tick_clock.global_clock})
        )
        nc.all_engine_barrier()
        assert self.sems is not None
        popped = nc._tile_sem_poison_stack.pop()
        assert popped is self._sem_poison
        nc.clear_and_free_semaphores(list(self.sems.allocated().values()))
        nc.all_engine_barrier()
        _split_all_waits(nc)

    tile.TileContext._drain_and_barrier = _drain_and_barrier


# ---------------------------------------------------------------------------
# Problem constants (hardcoded per the harness contract).
# ---------------------------------------------------------------------------
T, H, E, I = 1024, 2048, 16, 1408
TWO_I = 2 * I
N_CORES = 8
EPC = E // N_CORES               # experts per core = 2
TOP_K = 4
KH = H // 128                    # 16 h-tiles
KI = I // 128                    # 11 i-tiles (down contraction)
MI = TWO_I // 128                # 22 i-tiles per expert (11 gate + 11 up)
S_REAL = TWO_I // N_CORES        # 352 shared-intermediate channels per core
S_PAD = 384                      # padded to 3 tiles of 128
MS = 2 * S_PAD // 128            # 6 m-tiles for shared gate+up
KS = S_PAD // 128                # 3 k-tiles for shared down
TS = 512                         # shared-expert token slab (PSUM bank = 512 f32)

F32 = mybir.dt.float32
BF16 = mybir.dt.bfloat16


def _build_nc(C0, C1):
    _apply_tile_patch()
    nc = bass.Bass()
    CS = (C0, C1)

    # slab-major: [128, (th, k, t)] so the first gate group only needs slab 0
    xs_t = nc.declare_dram_parameter("xs_t", [128, 2 * KH * TS], BF16, isOutput=False)
    xe0_t = nc.declare_dram_parameter("xe0_t", [128, KH * C0], BF16, isOutput=False)
    xe1_t = nc.declare_dram_parameter("xe1_t", [128, KH * C1], BF16, isOutput=False)
    wgu_p = nc.declare_dram_parameter("wgu_p", [EPC, MI, 128, KH * 128], BF16, isOutput=False)
    wd_p = nc.declare_dram_parameter("wd_p", [EPC, KH, 128, KI * 128], BF16, isOutput=False)
    wsg_p = nc.declare_dram_parameter("wsg_p", [MS, 128, KH * 128], BF16, isOutput=False)
    wsd_p = nc.declare_dram_parameter("wsd_p", [128, KH * KS * 128], BF16, isOutput=False)
    ye0_t = nc.declare_dram_parameter("ye0_t", [128, KH * C0], BF16, isOutput=True)
    ye1_t = nc.declare_dram_parameter("ye1_t", [128, KH * C1], BF16, isOutput=True)
    ys_t = nc.declare_dram_parameter("ys_t", [2, 128, KH * TS], BF16, isOutput=True)
    xe_t = (xe0_t, xe1_t)
    ye_t = (ye0_t, ye1_t)

    with tile.TileContext(nc) as tc:
        with tc.tile_pool(name="xp", bufs=1) as xp, \
             tc.tile_pool(name="xep", bufs=1) as xep, \
             tc.tile_pool(name="wg", bufs=5) as wg, \
             tc.tile_pool(name="wdp", bufs=6) as wdp, \
             tc.tile_pool(name="gap", bufs=1) as gap, \
             tc.tile_pool(name="tmp", bufs=2) as tmp, \
             tc.tile_pool(name="outp", bufs=1) as outp, \
             tc.tile_pool(name="psE", bufs=3, space="PSUM") as psE, \
             tc.tile_pool(name="psY", bufs=2, space="PSUM") as psY, \
             tc.tile_pool(name="psS", bufs=3, space="PSUM") as psS:

            # ---- phase 1: shared gate/up (DMA-light; lets the weight
            #      stream bank a deep prefetch lead for the expert phases).
            #      All 6 wsg tiles are resident; their DMAs interleave with
            #      the xst chunks in consumption order. ----
            xst = xp.tile([128, 2 * KH * TS], BF16, tag="xs")
            chunk = 2 * KH * TS // 4
            wsg_t = [None] * MS

            def _wsg_load(i):
                t = xp.tile([128, KH * 128], BF16, tag=f"wsg{i}")
                nc.sync.dma_start(out=t[:], in_=wsg_p[i])
                wsg_t[i] = t

            def _chunk_load(q):
                nc.sync.dma_start(
                    out=xst[:, q * chunk:(q + 1) * chunk],
                    in_=xs_t[:, q * chunk:(q + 1) * chunk],
                )

            _wsg_load(0)
            _chunk_load(0)
            _wsg_load(KS)
            _chunk_load(1)
            _wsg_load(1)
            _wsg_load(KS + 1)
            _chunk_load(2)
            _wsg_load(2)
            _wsg_load(KS + 2)
            _chunk_load(3)
            # shared-down weights ride the phase-1 DMA slack and stay resident
            wsd = xp.tile([128, KH * KS * 128], BF16, tag="wsd")
            nc.sync.dma_start(out=wsd[:], in_=wsd_p[:])

            a_s = [[], []]
            for j in range(KS):
                wt_g = wsg_t[j]
                wt_u = wsg_t[KS + j]
                for th in range(2):
                    base = th * KH * TS
                    ps_g = psS.tile([128, TS], F32, tag="psS")
                    for k in range(KH):
                        nc.tensor.matmul(
                            out=ps_g[:], lhsT=wt_g[:, k * 128:(k + 1) * 128],
                            rhs=xst[:, base + k * TS:base + (k + 1) * TS],
                            start=(k == 0), stop=(k == KH - 1),
                        )
                    gt = tmp.tile([128, TS], F32, tag="gts")
                    nc.scalar.activation(
                        out=gt[:], in_=ps_g[:], func=mybir.ActivationFunctionType.Silu
                    )
                    ps_u = psS.tile([128, TS], F32, tag="psS")
                    for k in range(KH):
                        nc.tensor.matmul(
                            out=ps_u[:], lhsT=wt_u[:, k * 128:(k + 1) * 128],
                            rhs=xst[:, base + k * TS:base + (k + 1) * TS],
                            start=(k == 0), stop=(k == KH - 1),
                        )
                    at = gap.tile([128, TS], BF16, tag=f"as{th}_{j}")
                    nc.vector.tensor_tensor(
                        out=at[:], in0=ps_u[:], in1=gt[:], op=mybir.AluOpType.mult
                    )
                    a_s[th].append(at)

            # ---- phase 2: routed experts (small slot first so the heavier
            #      second GU phase, which also carries its xe DMA, is longer) --
            xe = []
            a = [[], []]
            for e in range(EPC):
                C = CS[e]
                xet = xep.tile([128, KH * C], BF16, tag=f"xe{e}")
                nc.sync.dma_start(out=xet[:], in_=xe_t[e][:])
                xe.append(xet)

                for j in range(KI):
                    wt_g = wg.tile([128, KH * 128], BF16, tag="wg")
                    nc.sync.dma_start(out=wt_g[:], in_=wgu_p[e, j])
                    ps_g = psE.tile([128, C], F32, tag="psE")
                    for k in range(KH):
                        nc.tensor.matmul(
                            out=ps_g[:], lhsT=wt_g[:, k * 128:(k + 1) * 128],
                            rhs=xet[:, k * C:(k + 1) * C],
                            start=(k == 0), stop=(k == KH - 1),
                        )
                    gt = tmp.tile([128, C], F32, tag="gt")
                    nc.scalar.activation(
                        out=gt[:], in_=ps_g[:], func=mybir.ActivationFunctionType.Silu
                    )
                    wt_u = wg.tile([128, KH * 128], BF16, tag="wg")
                    nc.sync.dma_start(out=wt_u[:], in_=wgu_p[e, KI + j])
                    ps_u = psE.tile([128, C], F32, tag="psE")
                    for k in range(KH):
                        nc.tensor.matmul(
                            out=ps_u[:], lhsT=wt_u[:, k * 128:(k + 1) * 128],
                            rhs=xet[:, k * C:(k + 1) * C],
                            start=(k == 0), stop=(k == KH - 1),
                        )
                    at = gap.tile([128, C], BF16, tag=f"a{e}_{j}")
                    nc.vector.tensor_tensor(
                        out=at[:], in0=ps_u[:], in1=gt[:], op=mybir.AluOpType.mult
                    )
                    a[e].append(at)

                ot = outp.tile([128, KH * C], BF16, tag=f"ot{e}")
                for m in range(KH):
                    wt_d = wdp.tile([128, KI * 128], BF16, tag="wd")
                    nc.sync.dma_start(out=wt_d[:], in_=wd_p[e, m])
                    ps_y = psY.tile([128, TS], F32, tag="psY")
                    for k in range(KI):
                        nc.tensor.matmul(
                            out=ps_y[:, :C], lhsT=wt_d[:, k * 128:(k + 1) * 128],
                            rhs=a[e][k][:],
                            start=(k == 0), stop=(k == KI - 1),
                        )
                    if m % 2 == 0:
                        nc.vector.tensor_copy(
                            out=ot[:, m * C:(m + 1) * C], in_=ps_y[:, :C]
                        )
                    else:
                        nc.scalar.copy(
                            out=ot[:, m * C:(m + 1) * C], in_=ps_y[:, :C]
                        )
                nc.sync.dma_start(out=ye_t[e][:], in_=ot[:])

            # ---- phase 3: shared down ----
            for th in range(2):
                yst = outp.tile([128, KH * TS], BF16, tag=f"ys{th}")
                for m in range(KH):
                    ps_y = psY.tile([128, TS], F32, tag="psY")
                    for k in range(KS):
                        base = m * KS * 128 + k * 128
                        nc.tensor.matmul(
                            out=ps_y[:], lhsT=wsd[:, base:base + 128],
                            rhs=a_s[th][k][:],
                            start=(k == 0), stop=(k == KS - 1),
                        )
                    if m % 2 == 0:
                        nc.vector.tensor_copy(
                            out=yst[:, m * TS:(m + 1) * TS], in_=ps_y[:]
                        )
                    else:
                        nc.scalar.copy(
                            out=yst[:, m * TS:(m + 1) * TS], in_=ps_y[:]
                        )
                    if m == KH // 2 - 1:
                        nc.sync.dma_start(
                            out=ys_t[th][:, :KH * TS // 2],
                            in_=yst[:, :KH * TS // 2],
                        )
                nc.sync.dma_start(
                    out=ys_t[th][:, KH * TS // 2:], in_=yst[:, KH * TS // 2:]
                )

    return nc


# ---------------------------------------------------------------------------
# Host side: routing, dispatch packing, gather/combine.
# ---------------------------------------------------------------------------


def _route(x, gate_w):
    """Exact fp32 router: softmax over expert logits, top-4, renormalize."""
    logits = x.astype(np.float32) @ gate_w.astype(np.float32).T      # [T, E]
    m = logits.max(-1, keepdims=True)
    p = np.exp(logits - m)
    p /= p.sum(-1, keepdims=True)
    top4 = np.argsort(-p, axis=-1, kind="stable")[:, :TOP_K]          # [T, K]
    w4 = np.take_along_axis(p, top4, axis=-1)
    w4 = w4 / w4.sum(-1, keepdims=True)
    return top4, w4


def _pack_core_weights(experts, w_gate_up_b, w_down_b, sgu_b, sd_b, c):
    """bf16 lhsT packing for one core's 2 experts + its shared-expert slice."""
    wgu = np.empty((EPC, MI, 128, KH * 128), bfloat16)
    wd = np.empty((EPC, KH, 128, KI * 128), bfloat16)
    for i, e in enumerate(experts):
        # GU lhsT per i-tile j: column block k holds W^T[k*128+h_in, j*128+i_in]
        wgu[i] = (
            w_gate_up_b[e].reshape(MI, 128, KH, 128).transpose(0, 3, 2, 1)
            .reshape(MI, 128, KH * 128)
        )
        # DOWN lhsT per h-tile m: column block k holds Wd^T[k*128+i_in, m*128+h_in]
        wd[i] = (
            w_down_b[e].reshape(KH, 128, KI, 128).transpose(0, 3, 2, 1)
            .reshape(KH, 128, KI * 128)
        )

    offs = S_REAL * c
    sg = np.zeros((2 * S_PAD, H), bfloat16)
    sg[:S_REAL] = sgu_b[offs:offs + S_REAL]
    sg[S_PAD:S_PAD + S_REAL] = sgu_b[TWO_I + offs:TWO_I + offs + S_REAL]
    wsg = (
        sg.reshape(MS, 128, KH, 128).transpose(0, 3, 2, 1)
        .reshape(MS, 128, KH * 128)
    )

    sd = np.zeros((S_PAD, H), bfloat16)
    sd[:S_REAL] = sd_b[:, offs:offs + S_REAL].T
    # [128(s_in), (m, k, h_in)] so slice m*KS*128 + k*128 is the (m, k) block
    wsd = (
        sd.reshape(KS, 128, KH, 128).transpose(1, 2, 0, 3)
        .reshape(128, KH * KS * 128)
    )
    return {
        "wgu_p": np.ascontiguousarray(wgu),
        "wd_p": np.ascontiguousarray(wd),
        "wsg_p": np.ascontiguousarray(wsg),
        "wsd_p": np.ascontiguousarray(wsd),
    }


_NC_CACHE = {}


def _get_nc(C0, C1):
    if (C0, C1) not in _NC_CACHE:
        _NC_CACHE[(C0, C1)] = _build_nc(C0, C1)
    return _NC_CACHE[(C0, C1)]


def _roundup4(n):
    return max(8, int(-(-n // 4) * 4))


def kernel(hidden_states, gate_w, w_gate_up, w_down, shared_gate_up, shared_down,
           _trace=False):
    x = np.asarray(hidden_states, np.float32)
    top4, w4 = _route(x, np.asarray(gate_w, np.float32))

    toks = [np.where(top4 == e)[0] for e in range(E)]
    kidx = [np.where(top4 == e)[1] for e in range(E)]
    counts = np.array([len(t) for t in toks])

    # pair heaviest expert with lightest so slot capacities hug actual loads;
    # slot 0 (processed first) is the light half
    order = np.argsort(-counts, kind="stable")
    slot0 = [int(order[2 * N_CORES - 1 - c]) for c in range(N_CORES)]  # small half
    slot1 = [int(order[c]) for c in range(N_CORES)]                    # big half
    C0 = _roundup4(max(counts[e] for e in slot0))
    C1 = _roundup4(max(counts[e] for e in slot1))

    nc = _get_nc(C0, C1)

    xb = x.astype(bfloat16)
    # [128, (th, k, t)]: slab-major x^T for the shared expert
    xs_t = np.ascontiguousarray(
        xb.T.reshape(KH, 128, 2, TS).transpose(1, 2, 0, 3).reshape(128, 2 * KH * TS)
    )
    wgu_b = np.asarray(w_gate_up, np.float32).astype(bfloat16)
    wd_b = np.asarray(w_down, np.float32).astype(bfloat16)
    sgu_b = np.asarray(shared_gate_up, np.float32).astype(bfloat16)
    sd_b = np.asarray(shared_down, np.float32).astype(bfloat16)

    def gathered(e, C):
        xg = np.zeros((C, H), bfloat16)
        xg[:counts[e]] = xb[toks[e]]
        return np.ascontiguousarray(
            xg.T.reshape(KH, 128, C).transpose(1, 0, 2).reshape(128, KH * C)
        )

    in_maps = []
    for c in range(N_CORES):
        m = _pack_core_weights((slot0[c], slot1[c]), wgu_b, wd_b, sgu_b, sd_b, c)
        m["xs_t"] = xs_t
        m["xe0_t"] = gathered(slot0[c], C0)
        m["xe1_t"] = gathered(slot1[c], C1)
        in_maps.append(m)

    res = run_bass_kernel_spmd(nc, in_maps, list(range(N_CORES)), trace=_trace)

    out = np.zeros((T, H), np.float32)
    ys = np.zeros((H, T), np.float32)
    for c, r in enumerate(res.results):
        yst = r["ys_t"].astype(np.float32)             # [2, 128, KH*TS]
        for th in range(2):
            ys[:, th * TS:(th + 1) * TS] += (
                yst[th].reshape(128, KH, TS).transpose(1, 0, 2).reshape(H, TS)
            )
        for i, e in enumerate((slot0[c], slot1[c])):
            n = counts[e]
            ye = r["ye0_t" if i == 0 else "ye1_t"].astype(np.float32)  # [128, KH*C]
            Ci = ye.shape[1] // KH
            y = ye.reshape(128, KH, Ci).transpose(1, 0, 2).reshape(H, Ci)[:, :n]
            w = w4[toks[e], kidx[e]].astype(np.float32)
            out[toks[e]] += (y * w).T
    out += ys.T
    if _trace:
        return out, res
    return out


# revision 3
# speedup vs baseline: 1.1477x; 1.1477x over previous
"""BailingMoE (top-4 of 16 experts + shared expert) on 8 Trainium2 NeuronCores.

Strategy (expert-parallel, sparse dispatch V4):
  - The router (67 MFLOP, 0.01% of total work) runs on the host in exact fp32;
    tokens are dispatched host-side into per-expert compact buffers (the
    all-to-all "dispatch" leg of the expert-parallel recipe; the combine leg
    is the host scatter-add, matching the baseline's host all-reduce of the
    shared partials).
  - Each core owns 2 experts and 1/8 of the shared-expert intermediate dim.
    Experts are paired big-load + small-load onto cores so the two SPMD slot
    capacities C0 >= C1 hug the actual token loads (~4x less routed FLOPs
    than dense).
  - Everything runs in bf16 (1 PE cycle/row, half the HBM bytes of fp32);
    PSUM accumulation is fp32. Measured end-to-end rel err ~4e-3 (gate 2e-2).
  - The kernel is DMA/PE co-limited (~46MB in vs ~190us of PE), so all
    activation I/O is packed as [128, KH*C] monoliths whose DMA descriptors
    are 9-32KB per partition row (full HBM bandwidth), and weight streams are
    emitted in exact consumption order with deep prefetch rings.
  - Outputs: per-expert compact y^T (bf16) + the core's shared partial y^T
    (bf16). Host applies the renormalized top-4 combine weights during the
    scatter-add in fp32.
"""

import numpy as np
from ml_dtypes import bfloat16

import concourse.bass as bass
import concourse.mybir as mybir
import concourse.tile as tile
from concourse.bass_utils import run_bass_kernel_spmd

# ---------------------------------------------------------------------------
# Walrus in this container rejects >1 sem-wait condition per instruction
# ("Too many sync wait commands"). Engines run their streams in order, so
# excess waits are legal on same-engine NoOps inserted before the instruction.
# ---------------------------------------------------------------------------
_counter = [0]


def _make_wait_nop(template_inst, waits):
    _counter[0] += 1
    nop = mybir.InstNoOp(
        name=f"I-waitsplit-{_counter[0]}", ins=[], outs=[], bass_nofuse=True
    )
    nop.engine = template_inst.engine
    nop.debug = template_inst.debug
    nop.sync_info = mybir.SyncInfo(on_wait=list(waits), on_update=[])
    return nop


def _split_all_waits(nc):
    for bass_bb in nc.bb_map.values():
        insts = bass_bb.bb.instructions
        i = 0
        while i < len(insts):
            inst = insts[i]
            si = inst.sync_info
            if si is not None and len(si.on_wait) > 1:
                waits = list(si.on_wait)
                del si.on_wait[:]
                si.on_wait.append(waits[-1])
                for j, w in enumerate(waits[:-1]):
                    nop = _make_wait_nop(inst, [w])
                    nc.register_instruction(nop, overwrite=True)
                    insts.insert(i + j, nop)
                i += len(waits) - 1
            i += 1


_PATCHED = [False]


def _install_cc_hook_debug():
    """Surface compile-hook exceptions (PJRT reports them as an opaque
    CallFunctionObjArgs error otherwise)."""
    import traceback
    import concourse.bass2jax as b2j
    b2j.install_neuronx_cc_hook()
    try:
        import libneuronxla
    except ImportError:
        return
    if getattr(libneuronxla, "_kernel_dbg_wrapped", False):
        return
    real = libneuronxla.neuronx_cc

    def hook(*a, **k):
        try:
            return real(*a, **k)
        except BaseException:
            traceback.print_exc()
            raise

    libneuronxla.neuronx_cc = hook
    libneuronxla._kernel_dbg_wrapped = True
    b2j.install_neuronx_cc_hook = lambda: None


def _apply_tile_patch():
    if _PATCHED[0]:
        return
    _PATCHED[0] = True
    _install_cc_hook_debug()

    def _drain_and_barrier(self, tick_clock, wait_clock):
        nc = self.nc
        drain_inst = nc.sync.drain()
        wait_clock.add_sem_waits(
            drain_inst.ins, tile.ScopedClock({None: tick_clock.global_clock})
        )
        nc.all_engine_barrier()
        assert self.sems is not None
        popped = nc._tile_sem_poison_stack.pop()
        assert popped is self._sem_poison
        nc.clear_and_free_semaphores(list(self.sems.allocated().values()))
        nc.all_engine_barrier()
        _split_all_waits(nc)

    tile.TileContext._drain_and_barrier = _drain_and_barrier


# ---------------------------------------------------------------------------
# Problem constants (hardcoded per the harness contract).
# ---------------------------------------------------------------------------
T, H, E, I = 1024, 2048, 16, 1408
TWO_I = 2 * I
N_CORES = 8
EPC = E // N_CORES               # experts per core = 2
TOP_K = 4
KH = H // 128                    # 16 h-tiles
KI = I // 128                    # 11 i-tiles (down contraction)
MI = TWO_I // 128                # 22 i-tiles per expert (11 gate + 11 up)
S_REAL = TWO_I // N_CORES        # 352 shared-intermediate channels per core
S_PAD = 384                      # padded to 3 tiles of 128
MS = 2 * S_PAD // 128            # 6 m-tiles for shared gate+up
KS = S_PAD // 128                # 3 k-tiles for shared down
TS = 512                         # shared-expert token slab (PSUM bank = 512 f32)

F32 = mybir.dt.float32
BF16 = mybir.dt.bfloat16


def _build_nc(C0, C1):
    _apply_tile_patch()
    nc = bass.Bass()
    CS = (C0, C1)

    # slab-major: [128, (th, k, t)] so the first gate group only needs slab 0
    xs_t = nc.declare_dram_parameter("xs_t", [128, 2 * KH * TS], BF16, isOutput=False)
    xe0_t = nc.declare_dram_parameter("xe0_t", [128, KH * C0], BF16, isOutput=False)
    xe1_t = nc.declare_dram_parameter("xe1_t", [128, KH * C1], BF16, isOutput=False)
    wgu_p = nc.declare_dram_parameter("wgu_p", [EPC, MI, 128, KH * 128], BF16, isOutput=False)
    wd_p = nc.declare_dram_parameter("wd_p", [EPC, KH, 128, KI * 128], BF16, isOutput=False)
    wsg_p = nc.declare_dram_parameter("wsg_p", [MS, 128, KH * 128], BF16, isOutput=False)
    wsd_p = nc.declare_dram_parameter("wsd_p", [128, KH * KS * 128], BF16, isOutput=False)
    ye0_t = nc.declare_dram_parameter("ye0_t", [128, KH * C0], BF16, isOutput=True)
    ye1_t = nc.declare_dram_parameter("ye1_t", [128, KH * C1], BF16, isOutput=True)
    ys_t = nc.declare_dram_parameter("ys_t", [2, 128, KH * TS], BF16, isOutput=True)
    xe_t = (xe0_t, xe1_t)
    ye_t = (ye0_t, ye1_t)

    with tile.TileContext(nc) as tc:
        with tc.tile_pool(name="xp", bufs=1) as xp, \
             tc.tile_pool(name="xep", bufs=1) as xep, \
             tc.tile_pool(name="wg", bufs=5) as wg, \
             tc.tile_pool(name="wdp", bufs=6) as wdp, \
             tc.tile_pool(name="gap", bufs=1) as gap, \
             tc.tile_pool(name="tmp", bufs=2) as tmp, \
             tc.tile_pool(name="outp", bufs=1) as outp, \
             tc.tile_pool(name="psE", bufs=2, space="PSUM") as psE, \
             tc.tile_pool(name="psY", bufs=3, space="PSUM") as psY, \
             tc.tile_pool(name="psS", bufs=3, space="PSUM") as psS:

            # ---- phase 1: shared gate/up (DMA-light; lets the weight
            #      stream bank a deep prefetch lead for the expert phases).
            #      All 6 wsg tiles are resident; their DMAs interleave with
            #      the xst chunks in consumption order. ----
            # PE pre-warm: the first ~16us are DMA-bound loading xst/wsg, and
            # the tensor engine only reaches its top p-state after ~3us of
            # continuous execution. Spin a throwaway accumulation group on
            # zeroed SBUF so the clock is ramped when real work arrives.
            warm = tmp.tile([128, TS], BF16, tag="warm")
            nc.vector.memset(warm[:], 0.0)
            ps_w = psY.tile([128, TS], F32, tag="psY")
            for i in range(20):
                nc.tensor.matmul(
                    out=ps_w[:], lhsT=warm[:, :128], rhs=warm[:],
                    start=(i == 0), stop=(i == 19),
                )

            xst = xp.tile([128, 2 * KH * TS], BF16, tag="xs")
            chunk = 2 * KH * TS // 4
            wsg_t = [None] * MS

            def _wsg_load(i):
                t = xp.tile([128, KH * 128], BF16, tag=f"wsg{i}")
                nc.sync.dma_start(out=t[:], in_=wsg_p[i])
                wsg_t[i] = t

            def _chunk_load(q):
                nc.sync.dma_start(
                    out=xst[:, q * chunk:(q + 1) * chunk],
                    in_=xs_t[:, q * chunk:(q + 1) * chunk],
                )

            _wsg_load(0)
            _chunk_load(0)
            _wsg_load(KS)
            _chunk_load(1)
            _wsg_load(1)
            _wsg_load(KS + 1)
            _chunk_load(2)
            _wsg_load(2)
            _wsg_load(KS + 2)
            _chunk_load(3)
            # shared-down weights ride the phase-1 DMA slack and stay resident
            wsd = xp.tile([128, KH * KS * 128], BF16, tag="wsd")
            nc.sync.dma_start(out=wsd[:], in_=wsd_p[:])

            a_s = [[], []]
            for j in range(KS):
                wt_g = wsg_t[j]
                wt_u = wsg_t[KS + j]
                for th in range(2):
                    base = th * KH * TS
                    ps_g = psS.tile([128, TS], F32, tag="psS")
                    for k in range(KH):
                        nc.tensor.matmul(
                            out=ps_g[:], lhsT=wt_g[:, k * 128:(k + 1) * 128],
                            rhs=xst[:, base + k * TS:base + (k + 1) * TS],
                            start=(k == 0), stop=(k == KH - 1),
                        )
                    gt = tmp.tile([128, TS], F32, tag="gts")
                    nc.scalar.activation(
                        out=gt[:], in_=ps_g[:], func=mybir.ActivationFunctionType.Silu
                    )
                    ps_u = psS.tile([128, TS], F32, tag="psS")
                    for k in range(KH):
                        nc.tensor.matmul(
                            out=ps_u[:], lhsT=wt_u[:, k * 128:(k + 1) * 128],
                            rhs=xst[:, base + k * TS:base + (k + 1) * TS],
                            start=(k == 0), stop=(k == KH - 1),
                        )
                    at = gap.tile([128, TS], BF16, tag=f"as{th}_{j}")
                    nc.vector.tensor_tensor(
                        out=at[:], in0=ps_u[:], in1=gt[:], op=mybir.AluOpType.mult
                    )
                    a_s[th].append(at)

            # ---- phase 2: routed experts (small slot first so the heavier
            #      second GU phase, which also carries its xe DMA, is longer) --
            xe = []
            a = [[], []]
            for e in range(EPC):
                C = CS[e]
                xet = xep.tile([128, KH * C], BF16, tag=f"xe{e}")
                nc.sync.dma_start(out=xet[:], in_=xe_t[e][:])
                xe.append(xet)

                for j in range(KI):
                    wt_g = wg.tile([128, KH * 128], BF16, tag="wg")
                    nc.sync.dma_start(out=wt_g[:], in_=wgu_p[e, j])
                    ps_g = psE.tile([128, C], F32, tag="psE")
                    for k in range(KH):
                        nc.tensor.matmul(
                            out=ps_g[:], lhsT=wt_g[:, k * 128:(k + 1) * 128],
                            rhs=xet[:, k * C:(k + 1) * C],
                            start=(k == 0), stop=(k == KH - 1),
                        )
                    gt = tmp.tile([128, C], F32, tag="gt")
                    nc.scalar.activation(
                        out=gt[:], in_=ps_g[:], func=mybir.ActivationFunctionType.Silu
                    )
                    wt_u = wg.tile([128, KH * 128], BF16, tag="wg")
                    nc.sync.dma_start(out=wt_u[:], in_=wgu_p[e, KI + j])
                    ps_u = psE.tile([128, C], F32, tag="psE")
                    for k in range(KH):
                        nc.tensor.matmul(
                            out=ps_u[:], lhsT=wt_u[:, k * 128:(k + 1) * 128],
                            rhs=xet[:, k * C:(k + 1) * C],
                            start=(k == 0), stop=(k == KH - 1),
                        )
                    at = gap.tile([128, C], BF16, tag=f"a{e}_{j}")
                    nc.vector.tensor_tensor(
                        out=at[:], in0=ps_u[:], in1=gt[:], op=mybir.AluOpType.mult
                    )
                    a[e].append(at)

                ot = outp.tile([128, KH * C], BF16, tag=f"ot{e}")
                for m in range(KH):
                    wt_d = wdp.tile([128, KI * 128], BF16, tag="wd")
                    nc.sync.dma_start(out=wt_d[:], in_=wd_p[e, m])
                    ps_y = psY.tile([128, TS], F32, tag="psY")
                    for k in range(KI):
                        nc.tensor.matmul(
                            out=ps_y[:, :C], lhsT=wt_d[:, k * 128:(k + 1) * 128],
                            rhs=a[e][k][:],
                            start=(k == 0), stop=(k == KI - 1),
                        )
                    if m % 2 == 0:
                        nc.vector.tensor_copy(
                            out=ot[:, m * C:(m + 1) * C], in_=ps_y[:, :C]
                        )
                    else:
                        nc.scalar.copy(
                            out=ot[:, m * C:(m + 1) * C], in_=ps_y[:, :C]
                        )
                nc.sync.dma_start(out=ye_t[e][:], in_=ot[:])

            # ---- phase 3: shared down ----
            for th in range(2):
                yst = outp.tile([128, KH * TS], BF16, tag=f"ys{th}")
                for m in range(KH):
                    ps_y = psY.tile([128, TS], F32, tag="psY")
                    for k in range(KS):
                        base = m * KS * 128 + k * 128
                        nc.tensor.matmul(
                            out=ps_y[:], lhsT=wsd[:, base:base + 128],
                            rhs=a_s[th][k][:],
                            start=(k == 0), stop=(k == KS - 1),
                        )
                    if m % 2 == 0:
                        nc.vector.tensor_copy(
                            out=yst[:, m * TS:(m + 1) * TS], in_=ps_y[:]
                        )
                    else:
                        nc.scalar.copy(
                            out=yst[:, m * TS:(m + 1) * TS], in_=ps_y[:]
                        )
                    if m % 4 == 3:
                        q = m // 4
                        nc.sync.dma_start(
                            out=ys_t[th][:, q * 4 * TS:(q + 1) * 4 * TS],
                            in_=yst[:, q * 4 * TS:(q + 1) * 4 * TS],
                        )

    return nc


# ---------------------------------------------------------------------------
# Host side: routing, dispatch packing, gather/combine.
# ---------------------------------------------------------------------------


def _route(x, gate_w):
    """Exact fp32 router: softmax over expert logits, top-4, renormalize."""
    logits = x.astype(np.float32) @ gate_w.astype(np.float32).T      # [T, E]
    m = logits.max(-1, keepdims=True)
    p = np.exp(logits - m)
    p /= p.sum(-1, keepdims=True)
    top4 = np.argsort(-p, axis=-1, kind="stable")[:, :TOP_K]          # [T, K]
    w4 = np.take_along_axis(p, top4, axis=-1)
    w4 = w4 / w4.sum(-1, keepdims=True)
    return top4, w4


def _pack_core_weights(experts, w_gate_up_b, w_down_b, sgu_b, sd_b, c):
    """bf16 lhsT packing for one core's 2 experts + its shared-expert slice."""
    wgu = np.empty((EPC, MI, 128, KH * 128), bfloat16)
    wd = np.empty((EPC, KH, 128, KI * 128), bfloat16)
    for i, e in enumerate(experts):
        # GU lhsT per i-tile j: column block k holds W^T[k*128+h_in, j*128+i_in]
        wgu[i] = (
            w_gate_up_b[e].reshape(MI, 128, KH, 128).transpose(0, 3, 2, 1)
            .reshape(MI, 128, KH * 128)
        )
        # DOWN lhsT per h-tile m: column block k holds Wd^T[k*128+i_in, m*128+h_in]
        wd[i] = (
            w_down_b[e].reshape(KH, 128, KI, 128).transpose(0, 3, 2, 1)
            .reshape(KH, 128, KI * 128)
        )

    offs = S_REAL * c
    sg = np.zeros((2 * S_PAD, H), bfloat16)
    sg[:S_REAL] = sgu_b[offs:offs + S_REAL]
    sg[S_PAD:S_PAD + S_REAL] = sgu_b[TWO_I + offs:TWO_I + offs + S_REAL]
    wsg = (
        sg.reshape(MS, 128, KH, 128).transpose(0, 3, 2, 1)
        .reshape(MS, 128, KH * 128)
    )

    sd = np.zeros((S_PAD, H), bfloat16)
    sd[:S_REAL] = sd_b[:, offs:offs + S_REAL].T
    # [128(s_in), (m, k, h_in)] so slice m*KS*128 + k*128 is the (m, k) block
    wsd = (
        sd.reshape(KS, 128, KH, 128).transpose(1, 2, 0, 3)
        .reshape(128, KH * KS * 128)
    )
    return {
        "wgu_p": np.ascontiguousarray(wgu),
        "wd_p": np.ascontiguousarray(wd),
        "wsg_p": np.ascontiguousarray(wsg),
        "wsd_p": np.ascontiguousarray(wsd),
    }


_NC_CACHE = {}


def _get_nc(C0, C1):
    if (C0, C1) not in _NC_CACHE:
        _NC_CACHE[(C0, C1)] = _build_nc(C0, C1)
    return _NC_CACHE[(C0, C1)]


def _roundup4(n):
    return max(8, int(-(-n // 4) * 4))


def kernel(hidden_states, gate_w, w_gate_up, w_down, shared_gate_up, shared_down,
           _trace=False):
    x = np.asarray(hidden_states, np.float32)
    top4, w4 = _route(x, np.asarray(gate_w, np.float32))

    toks = [np.where(top4 == e)[0] for e in range(E)]
    kidx = [np.where(top4 == e)[1] for e in range(E)]
    counts = np.array([len(t) for t in toks])

    # pair heaviest expert with lightest so slot capacities hug actual loads;
    # slot 0 (processed first) is the light half
    order = np.argsort(-counts, kind="stable")
    slot0 = [int(order[2 * N_CORES - 1 - c]) for c in range(N_CORES)]  # small half
    slot1 = [int(order[c]) for c in range(N_CORES)]                    # big half
    C0 = _roundup4(max(counts[e] for e in slot0))
    C1 = _roundup4(max(counts[e] for e in slot1))

    nc = _get_nc(C0, C1)

    xb = x.astype(bfloat16)
    # [128, (th, k, t)]: slab-major x^T for the shared expert
    xs_t = np.ascontiguousarray(
        xb.T.reshape(KH, 128, 2, TS).transpose(1, 2, 0, 3).reshape(128, 2 * KH * TS)
    )
    wgu_b = np.asarray(w_gate_up, np.float32).astype(bfloat16)
    wd_b = np.asarray(w_down, np.float32).astype(bfloat16)
    sgu_b = np.asarray(shared_gate_up, np.float32).astype(bfloat16)
    sd_b = np.asarray(shared_down, np.float32).astype(bfloat16)

    def gathered(e, C):
        xg = np.zeros((C, H), bfloat16)
        xg[:counts[e]] = xb[toks[e]]
        return np.ascontiguousarray(
            xg.T.reshape(KH, 128, C).transpose(1, 0, 2).reshape(128, KH * C)
        )

    in_maps = []
    for c in range(N_CORES):
        m = _pack_core_weights((slot0[c], slot1[c]), wgu_b, wd_b, sgu_b, sd_b, c)
        m["xs_t"] = xs_t
        m["xe0_t"] = gathered(slot0[c], C0)
        m["xe1_t"] = gathered(slot1[c], C1)
        in_maps.append(m)

    res = run_bass_kernel_spmd(nc, in_maps, list(range(N_CORES)), trace=_trace)

    out = np.zeros((T, H), np.float32)
    ys = np.zeros((H, T), np.float32)
    for c, r in enumerate(res.results):
        yst = r["ys_t"].astype(np.float32)             # [2, 128, KH*TS]
        for th in range(2):
            ys[:, th * TS:(th + 1) * TS] += (
                yst[th].reshape(128, KH, TS).transpose(1, 0, 2).reshape(H, TS)
            )
        for i, e in enumerate((slot0[c], slot1[c])):
            n = counts[e]
            ye = r["ye0_t" if i == 0 else "ye1_t"].astype(np.float32)  # [128, KH*C]
            Ci = ye.shape[1] // KH
            y = ye.reshape(128, KH, Ci).transpose(1, 0, 2).reshape(H, Ci)[:, :n]
            w = w4[toks[e], kidx[e]].astype(np.float32)
            out[toks[e]] += (y * w).T
    out += ys.T
    if _trace:
        return out, res
    return out
